# revision 15
# baseline (speedup 1.0000x reference)
"""DeepFM forward on 8 Trainium2 NeuronCores (Bass/Tile), v2.

Strategy: data-parallel over the batch (2048 samples/core). The fm1 +
embedding tables are concatenated into one [F*V, 17] fp32 table
replicated to every core and gathered on-device with vector-indirect
DMAs (128 rows/instruction — the HW cap: one index per partition, the
out free size is a contiguous block per index). The 416 gather
instructions round-robin over all 4 SWDGE queues.

v2 restructures everything after the gather:
 - The gathered [128, 26, 17] tile columns are PE-transposed DIRECTLY
   into feature-major bf16 activation chunks (no intermediate packed
   copy). Four 128-col transposes per (sample-group, chunk) batch into
   one PSUM bank, drained by one Activation copy.
 - The first-order term (sum of fm1 + X_dense @ Wd + bd + b3 + b1) is
   folded into an extra output column of a host-packed W1
   [456, 401]: fm1 table columns get 1.0, dense rows get Wd, and a
   host-added all-ones activation row carries the biases. y_1st is
   extracted from the z1 PSUM before BatchNorm; the junk BN channel is
   neutralized with zero gamma/beta and a zero row in W2.
 - BatchNorm batch statistics are computed with single-pass DVE
   bn_stats on the matmul PSUM + bn_aggr, globalized with one small
   AllGather per layer (15us constant vs 28us for AllReduce in the
   collective model) and combined exactly (equal per-core counts).
 - DNN matmuls run in bf16 (1 cycle/row on PE), W1 matmuls are
   software-pipelined against the per-sample-group transposes so PE
   starts as soon as the first sample group's gathers land.
 - FM second-order stats (sum-of-squares, |sum|^2, pre-scaled by 0.5
   via the activation scale) run during the AllGather windows.

Self-contained: hardcodes all shapes from the problem spec.
"""

import numpy as np

import concourse.bass as bass
import concourse.mybir as mybir
import concourse.tile as tile
from concourse.bass_utils import run_bass_kernel_spmd
from concourse.masks import make_identity

B, F, V, K, D, H = 16384, 26, 100000, 16, 13, 400
E = K + 1            # 17 floats per table row (16 emb + 1 fm1)
GW = F * E           # 442 gathered columns per sample
NCORES = 8
BL = B // NCORES     # 2048 samples per core
NT = BL // 128       # 16 sample tiles of 128
NSG = BL // 512      # 4 sample groups of 512
EPS = 1e-5
FP = mybir.dt.float32
BF = mybir.dt.bfloat16
RSQRT2 = 0.7071067811865476

# x^T chunk rows: 3*128 g-columns + (58 g-cols + 13 dense + 1 ones)
KC1 = [128, 128, 128, 72]
CW1 = [128, 128, 128, 58]      # g-columns per chunk
# z1 column chunks: 400 hidden + 1 y1st channel (chunk3 row 16)
HC1 = [128, 128, 128, 17]
H1P = 417
# h1 chunk rows for the W2 contraction (junk ch + ones row in chunk3)
KC2 = [128, 128, 128, 18]
# z2 column chunks: plain 400 hidden
HC2 = [128, 128, 128, 16]

NQ = 4  # SWDGE queues; gathers round-robin across them

_cached = {}


def _split_multi_waits(nc, max_waits=1):
    """This walrus build rejects instructions carrying >1 semaphore wait.
    Re-emit extra waits as standalone single-wait sem nops on the same
    engine immediately before the instruction (same per-engine order, so
    semantics are unchanged)."""
    for bb in nc.main_func.blocks:
        insts = bb.instructions
        new_list = []
        changed = False
        for inst in insts:
            si = inst.sync_info
            waits = list(si.on_wait) if si is not None and si.on_wait else []
            sem_waits = [w for w in waits if w.wait_reg is None]
            reg_waits = [w for w in waits if w.wait_reg is not None]
            if len(waits) > max_waits and sem_waits:
                keep = max(0, max_waits - len(reg_waits))
                move = sem_waits[: len(sem_waits) - keep]
                kept = sem_waits[len(sem_waits) - keep:]
                for w in move:
                    nop = mybir.InstEventSemaphore(
                        name=nc.get_next_instruction_name(),
                        engine=inst.engine,
                        ins=[], outs=[],
                        sync_info=mybir.SyncInfo(on_wait=[w], on_update=[]),
                    )
                    nc.register_instruction(nop, overwrite=True)
                    new_list.append(nop)
                    changed = True
                si.on_wait = reg_waits + kept
            new_list.append(inst)
        if changed:
            insts.clear()
            insts.extend(new_list)


def _build():
    nc = bass.Bass("TRN2", target_bir_lowering=False, debug=False,
                   num_devices=NCORES, dynamic_dma_scratch_size=65536,
                   num_swdge_queues=NQ)
    A = mybir.AluOpType
    AF = mybir.ActivationFunctionType

    tbl = nc.dram_tensor("tbl", [F * V, E], FP, kind="ExternalInput").ap()
    idxs = nc.dram_tensor("idxs", [128, NT * F], mybir.dt.int32,
                          kind="ExternalInput").ap()
    xdb = nc.dram_tensor("xdb", [D, BL], BF, kind="ExternalInput").ap()
    w1p = nc.dram_tensor("w1p", [456, 401], BF, kind="ExternalInput").ap()
    w2p = nc.dram_tensor("w2p", [402, 400], BF, kind="ExternalInput").ap()
    w3p = nc.dram_tensor("w3p", [400, 1], BF, kind="ExternalInput").ap()
    gb1 = nc.dram_tensor("gb1", [128, 8], FP, kind="ExternalInput").ap()
    gb2 = nc.dram_tensor("gb2", [128, 8], FP, kind="ExternalInput").ap()
    out_d = nc.dram_tensor("out", [128, NT], FP, kind="ExternalOutput").ap()

    with tile.TileContext(nc) as tc:
        with (
            tc.tile_pool(name="const", bufs=1) as cpool,
            tc.tile_pool(name="g", bufs=1) as gpool,
            tc.tile_pool(name="xt", bufs=1) as xtpool,
            tc.tile_pool(name="z", bufs=1) as zpool,
            tc.tile_pool(name="h", bufs=1) as hpool,
            tc.tile_pool(name="sm", bufs=4) as smpool,
            tc.tile_pool(name="ps", bufs=6, space="PSUM") as pspool,
            tc.tile_pool(name="pst", bufs=2, space="PSUM") as pstpool,
            tc.tile_pool(name="dram", bufs=1, space="DRAM") as dpool,
        ):
            # ---- constants / weights ----
            idxs_t = cpool.tile([128, NT * F], mybir.dt.int32)
            nc.sync.dma_start(out=idxs_t[:], in_=idxs[:, :])
            ident = cpool.tile([128, 128], FP)
            make_identity(nc, ident[:])
            w1_sb, w2_sb, w3_sb = [], [], []
            o1 = o2 = o3 = 0
            for kc in range(4):
                w1_c = cpool.tile([KC1[kc], 401], BF, tag=f"w1_{kc}")
                nc.sync.dma_start(out=w1_c[:], in_=w1p[o1:o1 + KC1[kc], :])
                w1_sb.append(w1_c)
                o1 += KC1[kc]
                w2_c = cpool.tile([KC2[kc], 400], BF, tag=f"w2_{kc}")
                nc.sync.dma_start(out=w2_c[:], in_=w2p[o2:o2 + KC2[kc], :])
                w2_sb.append(w2_c)
                o2 += KC2[kc]
                w3_c = cpool.tile([HC2[kc], 1], BF, tag=f"w3_{kc}")
                nc.sync.dma_start(out=w3_c[:], in_=w3p[o3:o3 + HC2[kc], :])
                w3_sb.append(w3_c)
                o3 += HC2[kc]
            gb1_sb = cpool.tile([128, 8], FP)
            nc.sync.dma_start(out=gb1_sb[:], in_=gb1[:, :])
            gb2_sb = cpool.tile([128, 8], FP)
            nc.sync.dma_start(out=gb2_sb[:], in_=gb2[:, :])
            eps_sb = cpool.tile([128, 1], FP)
            nc.vector.memset(eps_sb[:], EPS)
            one_sb = cpool.tile([1, 1], FP)
            nc.vector.memset(one_sb[:], 1.0)

            # ---- x^T chunks (feature-major bf16 activations) ----
            xt_sb = []
            for kc in range(3):
                xt_c = xtpool.tile([128, BL], BF, tag=f"xt{kc}")
                xt_sb.append(xt_c)
            xt3_c = xtpool.tile([72, BL], BF, tag="xt3")
            xt_sb.append(xt3_c)
            nc.sync.dma_start(out=xt_sb[3][58:71, :], in_=xdb[:, :])
            nc.vector.memset(xt_sb[3][71:72, :], 1.0)

            # FM per-sample accumulators (one column per sample tile)
            esq = cpool.tile([128, NT], FP)
            snorm = cpool.tile([128, NT], FP)

            # ---- gathers: 416 indirect DMAs, 4 queues ----
            g_all = gpool.tile([128, NT * F, E], FP)
            g2d = g_all[:].rearrange("p a b -> p (a b)")
            qnames = ["qPoolDynamic"] + [f"qPoolDynamic{i}" for i in range(1, NQ)]
            for j in range(NT * F):
                inst = nc.gpsimd.indirect_dma_start(
                    out=g2d[:, j * E:(j + 1) * E],
                    out_offset=None, in_=tbl[:],
                    in_offset=bass.IndirectOffsetOnAxis(
                        ap=idxs_t[:, j:j + 1], axis=0),
                )
                inst.ins.queue = qnames[j % NQ]

            # z1 chunks: bf16 except chunk3 (carries y1st, keep fp32)
            z1_sb = []
            for hc in range(3):
                z1_c = zpool.tile([128, BL], BF, tag=f"z1c{hc}")
                z1_sb.append(z1_c)
            z13_c = zpool.tile([HC1[3], BL], FP, tag="z1c3")
            z1_sb.append(z13_c)
            # layer-1 stats: uneven sample groups [4,4,4,3,1] tiles so the
            # last z1 drain after the final gather is tiny. bn_stats
            # emits (count, mean, count*var) for even/odd halves -> 10
            # triplets across 5 groups; combined manually with constant
            # count weights (exact for unequal groups, unlike bn_aggr).
            GT1 = [(0, 4), (4, 4), (8, 4), (12, 3), (15, 1)]
            NG1 = len(GT1)
            bnstA = cpool.tile([128, 4, NG1 * 6], FP)
            nc.vector.memset(bnstA[:].rearrange("p a b -> p (a b)"), 0.0)
            wtile = cpool.tile([128, 4, NG1 * 2], FP)
            for gi, (_, ntl) in enumerate(GT1):
                wv = ntl * 64.0 / BL          # (ntl*128/2) / 2048
                nc.vector.memset(wtile[:, :, 2 * gi:2 * gi + 1], wv)
                nc.vector.memset(wtile[:, :, 2 * gi + 1:2 * gi + 2], wv)

            def emit_transposes(grp):
                t0, ntl = grp
                for kc in range(4):
                    cw = CW1[kc]
                    pst = pstpool.tile([128, ntl * 128], FP, tag="pst")
                    for ts in range(ntl):
                        t = t0 + ts
                        base = t * GW + kc * 128
                        nc.tensor.transpose(
                            out=pst[:cw, ts * 128:(ts + 1) * 128],
                            in_=g2d[:, base:base + cw],
                            identity=ident[:])
                    nc.scalar.activation(
                        out=xt_sb[kc][:cw, t0 * 128:(t0 + ntl) * 128],
                        in_=pst[:cw, :], func=AF.Copy)

            def emit_w1(gi):
                t0, ntl = GT1[gi]
                c0, cn = t0 * 128, ntl * 128
                for hc in range(4):
                    hd = HC1[hc]
                    psz = pspool.tile([128, cn], FP, tag="psz")
                    for kc in range(4):
                        nc.tensor.matmul(
                            out=psz[:hd, :],
                            lhsT=w1_sb[kc][:, hc * 128:hc * 128 + hd],
                            rhs=xt_sb[kc][:KC1[kc], c0:c0 + cn],
                            start=(kc == 0), stop=(kc == 3))
                    nc.scalar.activation(
                        out=z1_sb[hc][:hd, c0:c0 + cn],
                        in_=psz[:hd, :], func=AF.Copy)
                    nc.vector.bn_stats(out=bnstA[:hd, hc, gi * 6:(gi + 1) * 6],
                                       in_=psz[:hd, :])

            emit_transposes(0)
            emit_transposes(1)
            emit_w1(0)
            emit_transposes(2)
            emit_w1(1)
            emit_transposes(3)
            emit_w1(2)
            emit_transposes(4)
            emit_w1(3)
            emit_w1(4)

            def ag_exchange(aggr, name):
                in_b = dpool.tile([128, 8], FP, tag=f"agin{name}")
                out_b = dpool.tile([NCORES, 128, 8], FP, tag=f"agout{name}")
                nc.sync.dma_start(out=in_b[:],
                                  in_=aggr[:].rearrange("p a b -> p (a b)"))
                nc.gpsimd.collective_compute(
                    "AllGather", A.bypass,
                    replica_groups=[list(range(NCORES))],
                    ins=[in_b.opt()], outs=[out_b.opt()])
                statg = cpool.tile([128, NCORES, 8], FP, tag=f"statg{name}")
                nc.sync.dma_start(
                    out=statg[:],
                    in_=out_b[:].rearrange("c p s -> p c s"))
                return statg

            def stats_allgather(bnst, hcs, name):
                """bn_aggr per chunk -> [128, 4, 2] -> AllGather"""
                aggr = cpool.tile([128, 4, 2], FP, tag=f"aggr{name}")
                nc.vector.memset(aggr[:].rearrange("p a b -> p (a b)"), 0.0)
                for hc in range(4):
                    hd = hcs[hc]
                    nc.vector.bn_aggr(
                        out=aggr[:hd, hc, :],
                        in_=bnst[hc][:hd, :, :].rearrange("p a b -> p (a b)"))
                return ag_exchange(aggr, name)

            def stats_weighted(name):
                """Exact per-core (mean, var) from the 10 uneven-group
                triplets: mean = sum w_t m_t; var = sum(M2_t)/n +
                sum w_t m_t^2 - mean^2, with w_t = c_t/n."""
                trip = bnstA[:].rearrange("p h (g f) -> p h f g", f=3)
                means = trip[:, :, 1, :]              # [128, 4, 2*NG1]
                m2s = trip[:, :, 2, :]
                mw = smpool.tile([128, 4, NG1 * 2], FP, tag="mw")
                nc.vector.tensor_tensor(out=mw[:], in0=means, in1=wtile[:],
                                        op=A.mult)
                aggr = cpool.tile([128, 4, 2], FP, tag=f"aggr{name}")
                meanc = aggr[:, :, 0:1].rearrange("p h x -> p (h x)")
                varc = aggr[:, :, 1:2].rearrange("p h x -> p (h x)")
                nc.vector.tensor_reduce(out=meanc, in_=mw[:],
                                        axis=mybir.AxisListType.X, op=A.add)
                msq = smpool.tile([128, 4, NG1 * 2], FP, tag="msq")
                nc.vector.tensor_tensor(out=msq[:], in0=means, in1=mw[:],
                                        op=A.mult)   # w * m^2
                m2w = cpool.tile([128, 4], FP, tag=f"m2w{name}")
                nc.vector.tensor_reduce(out=m2w[:], in_=msq[:],
                                        axis=mybir.AxisListType.X, op=A.add)
                m2sum = cpool.tile([128, 4], FP, tag=f"m2sum{name}")
                nc.vector.tensor_reduce(out=m2sum[:], in_=m2s,
                                        axis=mybir.AxisListType.X, op=A.add)
                mm2 = cpool.tile([128, 4], FP, tag=f"mm2{name}")
                nc.vector.tensor_tensor(out=mm2[:], in0=meanc, in1=meanc,
                                        op=A.mult)
                nc.vector.scalar_tensor_tensor(
                    out=varc, in0=m2sum[:], scalar=1.0 / BL, in1=m2w[:],
                    op0=A.mult, op1=A.add)
                nc.vector.tensor_tensor(out=varc, in0=varc, in1=mm2[:],
                                        op=A.subtract)
                return ag_exchange(aggr, name)

            def bn_params(statg, gb_sb, name):
                """Combine 8 cores' (mean, var) pairs exactly; return a, shift."""
                mv = cpool.tile([128, 8], FP, tag=f"mv{name}")
                nc.vector.tensor_reduce(
                    out=mv[:], in_=statg[:].rearrange("p c s -> p s c"),
                    axis=mybir.AxisListType.X, op=A.add)
                nc.vector.tensor_scalar_mul(mv[:], mv[:], 1.0 / NCORES)
                mvv = mv[:].rearrange("p (h m) -> p m h", m=2)
                gmean = mvv[:, 0, :]           # [128, 4] stride-2 view
                gvbar = mvv[:, 1, :]
                cmeans = statg[:].rearrange("p c (h m) -> p m c h", m=2)[:, 0, :, :]
                sqc = smpool.tile([128, NCORES, 4], FP, tag=f"sqc{name}")
                nc.vector.tensor_tensor(out=sqc[:], in0=cmeans, in1=cmeans,
                                        op=A.mult)
                m2b = cpool.tile([128, 4], FP, tag=f"m2b{name}")
                nc.vector.tensor_reduce(
                    out=m2b[:], in_=sqc[:].rearrange("p c h -> p h c"),
                    axis=mybir.AxisListType.X, op=A.add)
                nc.vector.tensor_scalar_mul(m2b[:], m2b[:], 1.0 / NCORES)
                mm = cpool.tile([128, 4], FP, tag=f"mm{name}")
                nc.vector.tensor_tensor(out=mm[:], in0=gmean, in1=gmean,
                                        op=A.mult)
                gvar = cpool.tile([128, 4], FP, tag=f"gvar{name}")
                nc.vector.tensor_tensor(out=gvar[:], in0=gvbar, in1=m2b[:],
                                        op=A.add)
                nc.vector.tensor_tensor(out=gvar[:], in0=gvar[:], in1=mm[:],
                                        op=A.subtract)
                std = cpool.tile([128, 4], FP, tag=f"std{name}")
                nc.scalar.activation(out=std[:], in_=gvar[:], func=AF.Sqrt,
                                     bias=eps_sb[:])
                rstd = cpool.tile([128, 4], FP, tag=f"rstd{name}")
                nc.vector.reciprocal(out=rstd[:], in_=std[:])
                a_sc = cpool.tile([128, 4], FP, tag=f"a{name}")
                nc.vector.tensor_tensor(out=a_sc[:], in0=gb_sb[:, 0:4],
                                        in1=rstd[:], op=A.mult)
                am = cpool.tile([128, 4], FP, tag=f"am{name}")
                nc.vector.tensor_tensor(out=am[:], in0=a_sc[:], in1=gmean,
                                        op=A.mult)
                shift = cpool.tile([128, 4], FP, tag=f"sh{name}")
                nc.vector.tensor_tensor(out=shift[:], in0=gb_sb[:, 4:8],
                                        in1=am[:], op=A.subtract)
                return a_sc, shift

            statg1 = stats_weighted("1")

            # ---- AllGather-window fillers: FM stats from the fp32 g tiles
            for t in range(NT):
                emb3d = g_all[:, t * F:(t + 1) * F, 0:K]
                sq = smpool.tile([128, F, K], FP, tag="sq")
                nc.scalar.activation(out=sq[:], in_=emb3d, func=AF.Square,
                                     scale=RSQRT2,
                                     accum_out=esq[:, t:t + 1])
                s_t = smpool.tile([128, K], FP, tag="s")
                nc.vector.tensor_reduce(
                    out=s_t[:], in_=emb3d.rearrange("p f k -> p k f"),
                    axis=mybir.AxisListType.X, op=A.add)
                ssq = smpool.tile([128, K], FP, tag="ssq")
                nc.scalar.activation(out=ssq[:], in_=s_t[:], func=AF.Square,
                                     scale=RSQRT2,
                                     accum_out=snorm[:, t:t + 1])
            # fm_y (pre-halved by the RSQRT2 activation scales)
            fmy = cpool.tile([128, NT], FP)
            nc.vector.tensor_tensor(out=fmy[:], in0=snorm[:], in1=esq[:],
                                    op=A.subtract)

            a1, sh1 = bn_params(statg1, gb1_sb, "1")

            # ---- layer 1 relu (h1 overwrites xt chunks), then layer 2 ----
            h1_sb = []
            for kc in range(3):
                h1_c = xtpool.tile([128, BL], BF, tag=f"xt{kc}")
                h1_sb.append(h1_c)
            h13_c = xtpool.tile([72, BL], BF, tag="xt3")
            h1_sb.append(h13_c)
            for hc in range(4):
                hd = HC1[hc]
                nc.scalar.activation(
                    out=h1_sb[hc][:hd, :], in_=z1_sb[hc][:hd, :],
                    func=AF.Relu, scale=a1[:hd, hc:hc + 1],
                    bias=sh1[:hd, hc:hc + 1])
            nc.vector.memset(h1_sb[3][17:18, :], 1.0)  # b2 ones row

            z2_sb = []
            for hc in range(3):
                z2_c = zpool.tile([128, BL], BF, tag=f"z1c{hc}")
                z2_sb.append(z2_c)
            z23_c = zpool.tile([HC2[3], BL], BF, tag="z2c3")
            z2_sb.append(z23_c)
            bnst2 = []
            for hc in range(4):
                bn2_c = cpool.tile([128, NSG, 6], FP, tag=f"bnst2_{hc}")
                bnst2.append(bn2_c)
            for sg in range(NSG):
                for hc in range(4):
                    hd = HC2[hc]
                    psz = pspool.tile([128, 512], FP, tag="psz")
                    for kc in range(4):
                        nc.tensor.matmul(
                            out=psz[:hd, :],
                            lhsT=w2_sb[kc][:, hc * 128:hc * 128 + hd],
                            rhs=h1_sb[kc][:KC2[kc], sg * 512:(sg + 1) * 512],
                            start=(kc == 0), stop=(kc == 3))
                    nc.scalar.activation(
                        out=z2_sb[hc][:hd, sg * 512:(sg + 1) * 512],
                        in_=psz[:hd, :], func=AF.Copy)
                    nc.vector.bn_stats(out=bnst2[hc][:hd, sg, :],
                                       in_=psz[:hd, :])

            statg2 = stats_allgather(bnst2, HC2, "2")
            a2, sh2 = bn_params(statg2, gb2_sb, "2")

            # ---- layer 2 relu -> h2 ----
            h2_sb = []
            for hc in range(3):
                h2_c = hpool.tile([128, BL], BF, tag=f"h2c{hc}")
                h2_sb.append(h2_c)
            h23_c = hpool.tile([HC2[3], BL], BF, tag="h2c3")
            h2_sb.append(h23_c)
            # ---- layer 2 relu (per sample group) + output head ----
            out_sb = cpool.tile([128, NT], FP)
            for sg in range(NSG):
                for hc in range(4):
                    hd = HC2[hc]
                    nc.scalar.activation(
                        out=h2_sb[hc][:hd, sg * 512:(sg + 1) * 512],
                        in_=z2_sb[hc][:hd, sg * 512:(sg + 1) * 512],
                        func=AF.Relu, scale=a2[:hd, hc:hc + 1],
                        bias=sh2[:hd, hc:hc + 1])
                psy = pstpool.tile([1, 512], FP, tag="pst")
                for kc in range(4):
                    nc.tensor.matmul(
                        out=psy[:, :], lhsT=w3_sb[kc][:],
                        rhs=h2_sb[kc][:HC2[kc], sg * 512:(sg + 1) * 512],
                        start=(kc == 0), stop=(kc == 3))
                ypre = smpool.tile([1, 512], FP, tag="ypre")
                nc.vector.tensor_tensor(
                    out=ypre[:], in0=psy[:],
                    in1=z1_sb[3][16:17, sg * 512:(sg + 1) * 512], op=A.add)
                for sub in range(4):
                    t = sg * 4 + sub
                    psf = pstpool.tile([128, 1], FP, tag="pst")
                    nc.tensor.matmul(
                        out=psf[:], lhsT=ypre[0:1, sub * 128:(sub + 1) * 128],
                        rhs=one_sb[:], start=True, stop=True)
                    nc.vector.tensor_tensor(out=out_sb[:, t:t + 1],
                                            in0=psf[:], in1=fmy[:, t:t + 1],
                                            op=A.add)
                nc.sync.dma_start(out=out_d[:, sg * 4:(sg + 1) * 4],
                                  in_=out_sb[:, sg * 4:(sg + 1) * 4])

    _split_multi_waits(nc)
    return nc


def _prep_core(c, X_cat, X_dense):
    sl = slice(c * BL, (c + 1) * BL)
    xc = np.asarray(X_cat[sl], dtype=np.int64)
    # idxs[p, t*F+f] = f*V + X_cat[c*BL + t*128 + p, f]
    gidx = (xc + np.arange(F, dtype=np.int64)[None, :] * V).astype(np.int32)
    idxs = np.ascontiguousarray(
        gidx.reshape(NT, 128, F).transpose(1, 0, 2).reshape(128, NT * F))
    import ml_dtypes
    xdb = np.ascontiguousarray(
        np.asarray(X_dense[sl], dtype=np.float32).T).astype(ml_dtypes.bfloat16)
    return {"idxs": idxs, "xdb": xdb}


def _pack_gb(g, b, n):
    out = np.zeros((128, 8), np.float32)
    gp = np.zeros(512, np.float32)
    bp = np.zeros(512, np.float32)
    gp[:n] = g
    bp[:n] = b
    out[:, 0:4] = gp.reshape(4, 128).T
    out[:, 4:8] = bp.reshape(4, 128).T
    return out


def kernel(X_cat, X_dense, fm1_tables, emb_tables, Wd, bd,
           W1, b1, g1, be1, W2, b2, g2, be2, W3, b3):
    import ml_dtypes
    if "nc" not in _cached:
        _cached["nc"] = _build()
    nc = _cached["nc"]

    tbl = np.concatenate(
        [np.asarray(emb_tables, np.float32).reshape(F * V, K),
         np.asarray(fm1_tables, np.float32).reshape(F * V, 1)],
        axis=1)
    tbl = np.ascontiguousarray(tbl)

    W1 = np.asarray(W1, np.float32)
    W2 = np.asarray(W2, np.float32)
    W3 = np.asarray(W3, np.float32)
    Wd = np.asarray(Wd, np.float32).reshape(D)
    b1 = np.asarray(b1, np.float32).reshape(H)
    b2 = np.asarray(b2, np.float32).reshape(H)
    bias = float(np.asarray(bd).reshape(-1)[0]) + \
        float(np.asarray(b3).reshape(-1)[0])

    # W1 packed [456, 401]: rows 0..441 are gathered g-columns (f, e);
    # rows 442..454 dense features; row 455 all-ones (biases).
    w1p = np.zeros((456, 401), np.float32)
    cols = np.arange(GW)
    fs, es = cols // E, cols % E
    emb_mask = es < K
    w1p[cols[emb_mask], :400] = W1[fs[emb_mask] * K + es[emb_mask], :]
    w1p[cols[~emb_mask], 400] = 1.0
    w1p[442:455, :400] = W1[F * K:, :]
    w1p[442:455, 400] = Wd
    w1p[455, :400] = b1
    w1p[455, 400] = bias

    # W2 packed [402, 400]: rows 0..399 = W2, 400 junk-channel zero,
    # 401 = b2 (ones row).
    w2p = np.zeros((402, 400), np.float32)
    w2p[0:400, :] = W2
    w2p[401, :] = b2

    shared = {
        "tbl": tbl,
        "w1p": w1p.astype(ml_dtypes.bfloat16),
        "w2p": w2p.astype(ml_dtypes.bfloat16),
        "w3p": np.ascontiguousarray(W3.reshape(400, 1)).astype(
            ml_dtypes.bfloat16),
        "gb1": _pack_gb(np.asarray(g1, np.float32),
                        np.asarray(be1, np.float32), H),
        "gb2": _pack_gb(np.asarray(g2, np.float32),
                        np.asarray(be2, np.float32), H),
    }
    in_maps = []
    for c in range(NCORES):
        m = dict(shared)
        m.update(_prep_core(c, X_cat, X_dense))
        in_maps.append(m)

    res = run_bass_kernel_spmd(nc, in_maps, core_ids=list(range(NCORES)))
    y = np.empty((B, 1), np.float32)
    for c in range(NCORES):
        o = res.results[c]["out"]            # [128, NT]
        y[c * BL:(c + 1) * BL, 0] = o.T.reshape(BL)
    return y
